# revision 5
# baseline (speedup 1.0000x reference)
"""Trainium2 Bass kernel for nn_Part_Block (SE-style dynamic-weight CNN block).

Computation (per batch b):
    pooled = mean_hw x[b]                       (C,)
    hidden = silu(pooled @ fc1_w.T + fc1_b)     (128,)
    dw     = (hidden @ fc2_w.T + fc2_b)         (P*C,) -> (P, C)
    base   = x[b] * conv_w + conv_b             (C, H, W)
    out    = softmax_p( einsum('chw,pc->phw', base, dw) )

Sharding: data-parallel over batch across the 8 cores (4 batches/core),
no collectives.  The depthwise conv is folded into the dynamic weights
(host-side SE path, 0.13% of FLOPs):
    logits[p,hw] = sum_c x[c,hw] * (conv_w[c]*dw[p,c]) + beta[p]
    beta[p]      = sum_c conv_b[c]*dw[p,c]

The executing backend charges a large, mostly-flat cost per *emitted
instruction*, so the kernel minimizes instruction count:
  - x5[part, b, t, f] = x[b, part*16 + t, f]: host transpose of 36KB
    contiguous blocks, shipped f32 in ONE contiguous DMA together with
    the per-(b,t) weight columns and the beta bias columns.
  - f32 matmuls are self-loading (no InstLdweights legalization pass),
    so the einsum is 128 PE instructions, not 256.
  - Einsum per batch: 16 accumulating K=128 matmuls for pixels 0..511
    plus 16 for the 64-pixel tail into a reused [4, 1024] f32 PSUM tile
    (M=4 output rows exactly match the nonzero rows).
  - Exp with per-partition beta bias writes e[p, b*576:...]; softmax
    over p is one gpsimd partition_all_reduce + reciprocal + multiply.
  - Raw BassBlock engine programs with explicit semaphores instead of
    the Tile framework (fewer sync/drain/branch instructions), and
    detect_race_conditions=False.
"""

import numpy as np

import concourse.bass as bass
import concourse.bass_isa as bass_isa
import concourse.mybir as mybir
from concourse import bacc
from concourse.bass_utils import run_bass_kernel_spmd

N_CORES = 8
B, C, H, W = 32, 2048, 24, 24
HW = H * W                      # 576
P = 4
B_LOC = B // N_CORES            # 4 batches per core
NT = C // 128                   # 16 channel tiles per batch
NMAIN = 512                     # pixels in the wide PSUM window

XOFF = 0
WOFF = B_LOC * NT * HW          # 36864 (x columns)
AOFF = WOFF + B_LOC * NT * P    # 37120 (weight columns end; beta cols)
CINW = AOFF + B_LOC             # 37124

F32 = mybir.dt.float32

_BUILD_CACHE: dict = {}


def _build(repeat: int = 1):
    """Build + compile the SPMD single-core program (same on all 8 cores)."""
    nc = bacc.Bacc(
        "TRN2", target_bir_lowering=False, debug=False, num_devices=N_CORES,
        detect_race_conditions=False,
    )
    cin_d = nc.dram_tensor("cin", [128, CINW], F32, kind="ExternalInput")
    ys = nc.dram_tensor("ys", [B_LOC, P, HW], F32, kind="ExternalOutput")

    cin = nc.alloc_sbuf_tensor("cin_sb", [128, CINW], F32)
    e_sb = nc.alloc_sbuf_tensor("e_sb", [P, B_LOC * HW], F32)
    s_sb = nc.alloc_sbuf_tensor("s_sb", [P, B_LOC * HW], F32)
    r_sb = nc.alloc_sbuf_tensor("r_sb", [P, B_LOC * HW], F32)
    out_sb = nc.alloc_sbuf_tensor("out_sb", [P, B_LOC * HW], F32)
    ps_e = nc.alloc_psum_tensor("ps_e", [P, 1024], F32)

    dma_sem = nc.alloc_semaphore("dma_sem")
    pe_sem = nc.alloc_semaphore("pe_sem")
    act_sem = nc.alloc_semaphore("act_sem")
    gp_sem = nc.alloc_semaphore("gp_sem")
    dv_sem = nc.alloc_semaphore("dv_sem")
    od_sem = nc.alloc_semaphore("od_sem")

    with nc.Block("main") as blk:

        @blk.sync
        def _(sync: bass.BassEngine):
            for r in range(repeat):
                sync.dma_start(cin[:], cin_d.ap()).then_inc(dma_sem, 16)
                sync.wait_ge(dv_sem, r + 1)
                sync.dma_start(
                    ys.ap().rearrange("b p f -> p b f"), out_sb[:]
                ).then_inc(od_sem, 16)
            sync.wait_ge(od_sem, 16 * repeat)

        @blk.tensor
        def _(pe: bass.BassEngine):
            for r in range(repeat):
                pe.wait_ge(dma_sem, 16 * (r + 1))
                for b in range(B_LOC):
                    if r + b > 0:
                        # previous batch's Exp must have consumed ps_e
                        pe.wait_ge(act_sem, 4 * r + b)
                    for t in range(NT):
                        base = (b * NT + t) * HW
                        lw = cin[:, WOFF + (b * NT + t) * P
                                 : WOFF + (b * NT + t + 1) * P]
                        pe.matmul(
                            ps_e[:, 0:NMAIN],
                            lhsT=lw,
                            rhs=cin[:, base : base + NMAIN],
                            start=(t == 0),
                            stop=(t == NT - 1),
                            skip_group_check=True,
                        )
                        mm = pe.matmul(
                            ps_e[:, NMAIN:HW],
                            lhsT=lw,
                            rhs=cin[:, base + NMAIN : base + HW],
                            start=(t == 0),
                            stop=(t == NT - 1),
                            skip_group_check=True,
                        )
                        if t == NT - 1:
                            mm.then_inc(pe_sem, 1)

        @blk.scalar
        def _(act: bass.BassEngine):
            for r in range(repeat):
                for b in range(B_LOC):
                    if b == 0 and r > 0:
                        # e_sb still read by previous iteration's multiply
                        act.wait_ge(dv_sem, r)
                    act.wait_ge(pe_sem, 4 * r + b + 1)
                    act.activation(
                        e_sb[:, b * HW : (b + 1) * HW], ps_e[:, 0:HW],
                        mybir.ActivationFunctionType.Exp,
                        bias=cin[0:P, AOFF + b : AOFF + b + 1],
                    ).then_inc(act_sem, 1)

        @blk.gpsimd
        def _(gp: bass.BassEngine):
            for r in range(repeat):
                gp.wait_ge(act_sem, 4 * (r + 1))
                gp.partition_all_reduce(
                    s_sb[:], e_sb[:], channels=P,
                    reduce_op=bass_isa.ReduceOp.add,
                ).then_inc(gp_sem, 1)

        @blk.vector
        def _(dv: bass.BassEngine):
            for r in range(repeat):
                dv.wait_ge(gp_sem, r + 1)
                if r > 0:
                    # out_sb still being written out by previous DMA
                    dv.wait_ge(od_sem, 16 * r)
                dv.reciprocal(r_sb[:], s_sb[:])
                dv.tensor_mul(out_sb[:], e_sb[:], r_sb[:]).then_inc(dv_sem, 1)

    nc.compile()
    return nc


def _host_se(x3, fc1_w, fc1_b, fc2_w, fc2_b, conv_w, conv_b):
    """SE squeeze path on host (tiny): dwp (B, P, C) and beta (B, P), f64."""
    pooled = x3.mean(axis=2, dtype=np.float64)                    # (B, C)
    z = pooled @ fc1_w.astype(np.float64).T + fc1_b.astype(np.float64)
    hidden = z / (1.0 + np.exp(-z))                               # silu
    dw = hidden @ fc2_w.astype(np.float64).T + fc2_b.astype(np.float64)
    dwp = dw.reshape(B, P, C) * conv_w.astype(np.float64)[None, None, :]
    beta = dw.reshape(B, P, C) @ conv_b.astype(np.float64)        # (B, P)
    return dwp, beta


def make_in_maps(x, fc1_w, fc1_b, fc2_w, fc2_b, conv_w, conv_b):
    x3 = np.asarray(x, np.float32).reshape(B, C, HW)
    dwp, beta = _host_se(
        x3,
        np.asarray(fc1_w, np.float32), np.asarray(fc1_b, np.float32),
        np.asarray(fc2_w, np.float32), np.asarray(fc2_b, np.float32),
        np.asarray(conv_w, np.float32), np.asarray(conv_b, np.float32),
    )
    in_maps = []
    for i in range(N_CORES):
        sl = slice(i * B_LOC, (i + 1) * B_LOC)
        cin = np.zeros((128, CINW), np.float32)
        # x5[part, b, t, f] = x3[b, part*16 + t, f]
        cin[:, XOFF:WOFF] = (
            x3[sl].reshape(B_LOC, 128, NT, HW).transpose(1, 0, 2, 3)
            .reshape(128, B_LOC * NT * HW))
        # wt[part, b, t, p] = dwp[b, p, part*16 + t]
        cin[:, WOFF:AOFF] = (
            dwp[sl].reshape(B_LOC, P, 128, NT).transpose(2, 0, 3, 1)
            .reshape(128, B_LOC * NT * P).astype(np.float32))
        # beta bias columns: cin[p, AOFF + b] = beta[b, p]
        cin[0:P, AOFF:CINW] = beta[sl].T.astype(np.float32)
        in_maps.append({"cin": cin})
    return in_maps


def _run(in_maps, repeat: int = 1):
    if repeat not in _BUILD_CACHE:
        _BUILD_CACHE[repeat] = _build(repeat)
    nc = _BUILD_CACHE[repeat]
    return run_bass_kernel_spmd(nc, in_maps, list(range(N_CORES)))


def kernel(x, fc1_w, fc1_b, fc2_w, fc2_b, conv_w, conv_b):
    in_maps = make_in_maps(x, fc1_w, fc1_b, fc2_w, fc2_b, conv_w, conv_b)
    res = _run(in_maps, repeat=1)
    out = np.concatenate(
        [res.results[i]["ys"] for i in range(N_CORES)], axis=0
    )
    return np.ascontiguousarray(out.reshape(B, P, H, W).astype(np.float32))
